# revision 1
# baseline (speedup 1.0000x reference)
"""Trainium2 Bass kernel: 6-head causal self-attention (nn_MultiHead).

Strategy: pure data-parallel over batch B=256 across 8 NeuronCores
(32 batches/core, no collectives). Per batch, on-chip layout keeps the
contraction dim on SBUF partitions everywhere:

  host:    x [B,T,D] -> xT [B,D,T] fp16;  W_qkv -> Wq/Wk/Wv [D, H*HS] fp16
  proj:    qT/kT [(h e), t] = W.T @ xT    (PE; two batches fused, N=512)
           v_aug [s, 6*65]  = xT.T @ Wv, + persistent ones column
  scores:  S^T [s, t] = kT_h^T @ qT_h per head (K=64); causal skip: the
           s-chunk-1 matmul only covers t>=128 (N=128)
  softmax: -1e9 causal mask added to the two diagonal blocks (DVE, one
           strided op); P = exp(S/8) on ACT -> fp16 [128, 384]
  PV:      O_aug [65, t] = V_aug^T @ P^T; row 64 = softmax denominators
  norm:    gather denom row (ACT) -> 1/x approx (DVE custom op, SBUF) ->
           partition_broadcast per pair (GPSIMD) -> O^T *= r (DVE)
  out:     y [t, d] = O^T.T @ W_out + b (PE + DVE), DMA out

The six (sub-batch, head-pair) attention stages run through a software
pipeline: S matmuls lead the softmax/PV block by one stage, and the last
stage's PV + output projection are deferred into the next macro-batch so
the PE never drains at batch boundaries. Matmul operands are fp16
(1 cycle/row streaming + fast weight load); accumulation is fp32 PSUM.
"""

import sys

import numpy as np

if "/opt/trn_rl_repo" not in sys.path:
    sys.path.insert(0, "/opt/trn_rl_repo")

from contextlib import ExitStack

import concourse.bass as bass  # noqa: F401
import concourse.tile as tile
from concourse import bacc, mybir
from concourse.bass_utils import run_bass_kernel_spmd

B, T, D, H, HS = 256, 256, 384, 6, 64
NCORES = 8
BPC = B // NCORES  # batches per core
F32 = mybir.dt.float32
F16 = mybir.dt.float16
EXP = mybir.ActivationFunctionType.Exp
SCALE = 1.0 / 8.0  # 1/sqrt(HS)


def _emit(ctx, tc, aps, bpc):
    nc = tc.nc
    xT, wq, wk, wv, wo, bias, msk, vone, y = aps
    assert bpc % 2 == 0
    nmb = bpc // 2  # macro-batches of 2

    singles = ctx.enter_context(tc.tile_pool(name="singles", bufs=1))
    xpool = ctx.enter_context(tc.tile_pool(name="xp", bufs=6))
    qkpool = ctx.enter_context(tc.tile_pool(name="qkp", bufs=12))
    ppool = ctx.enter_context(tc.tile_pool(name="pp", bufs=5))
    opool = ctx.enter_context(tc.tile_pool(name="op", bufs=12))
    rpool = ctx.enter_context(tc.tile_pool(name="rp", bufs=6))
    bcpool = ctx.enter_context(tc.tile_pool(name="bcp", bufs=4))
    ypool = ctx.enter_context(tc.tile_pool(name="yp", bufs=4))
    ps_work = ctx.enter_context(tc.tile_pool(name="ps_work", bufs=5, space="PSUM"))
    ps_o = ctx.enter_context(tc.tile_pool(name="ps_o", bufs=2, space="PSUM"))
    ps_y = ctx.enter_context(tc.tile_pool(name="ps_y", bufs=1, space="PSUM"))

    # Constants / weights, loaded once.
    def _load(name, src, shape, dt=F16):
        t = singles.tile(shape, dt, tag=name, name=name)
        nc.sync.dma_start(out=t, in_=src)
        return t

    wq_sb = [_load(f"wq{i}", wq[i * 128 : (i + 1) * 128, :], [128, D]) for i in range(3)]
    wk_sb = [_load(f"wk{i}", wk[i * 128 : (i + 1) * 128, :], [128, D]) for i in range(3)]
    wv_sb = [_load(f"wv{i}", wv[i * 128 : (i + 1) * 128, :], [128, D]) for i in range(3)]
    wo_sb = [_load(f"wo{i}", wo[i * 128 : (i + 1) * 128, :], [128, D]) for i in range(3)]
    bias_sb = _load("bias", bias, [128, D], dt=F32)
    msk_sb = _load("msk", msk, [128, 256], dt=F32)
    msk3 = msk_sb.rearrange("p (a b) -> p a b", b=128)

    # Persistent v_aug tiles [macro-parity][sub-batch][s-tile]: ones columns
    # are DMA'd once and survive all batches (the per-batch copy writes only
    # cols 0:64 of each 65-wide head block).
    va_all = []
    for par in range(2):
        subs = []
        for sub in range(2):
            pair = []
            for st in range(2):
                t = singles.tile(
                    [128, H * 65], F16, tag=f"va{par}{sub}{st}", name=f"va{par}{sub}{st}"
                )
                nc.sync.dma_start(
                    out=t.rearrange("p (h c) -> p h c", c=65)[:, :, 64:65], in_=vone
                )
                pair.append(t)
            subs.append(pair)
        va_all.append(subs)

    # xT viewed so two consecutive batches concatenate along the free dim:
    # [mb, d, (sub t)] per 128-row d-chunk
    def x2_src(mb, kc):
        return xT[2 * mb : 2 * mb + 2, kc * 128 : (kc + 1) * 128, :].rearrange(
            "b d t -> d b t"
        )

    pv_q = []
    out_q = []

    def emit_scores(qk_sb, sub, hp):
        qt = qk_sb[("q", hp)]
        kt = qk_sb[("k", hp)]
        toff = sub * T
        sps = [ps_work.tile([128, 2 * T], F32, tag="work", name="s_ps") for _ in range(2)]
        # s-chunk 0: full t (N=256); adjacent MMs use row groups 0/64
        for hh in range(2):
            nc.tensor.matmul(
                sps[hh][:, 0:T],
                kt[hh * 64 : (hh + 1) * 64, toff : toff + 128],
                qt[hh * 64 : (hh + 1) * 64, toff : toff + T],
                start=True,
                stop=True,
            )
        # s-chunk 1: only t >= 128 (N=128) -- the rest is causally masked
        for hh in range(2):
            nc.tensor.matmul(
                sps[hh][:, T : T + 128],
                kt[hh * 64 : (hh + 1) * 64, toff + 128 : toff + T],
                qt[hh * 64 : (hh + 1) * 64, toff + 128 : toff + T],
                start=True,
                stop=True,
            )
        return sps

    def emit_softmax_pv(va_pair, oT_sb, hp, sps):
        o_pair = ps_o.tile([65, 2 * T], F32, tag="o", name="o_ps")
        for hh in range(2):
            h = hp * 2 + hh
            sp = sps[hh]
            p_sb = ppool.tile([128, 384], F16, tag="p", name="p_sb")
            # additive -1e9 causal mask on the diagonal blocks (cols 0:128
            # and 256:384 of sp) in one strided DVE add
            spd = sp.rearrange("p (a b) -> p a b", b=128)[:, 0:3:2, :]
            nc.vector.tensor_add(spd, spd, msk3)
            nc.scalar.activation(p_sb, sp[:, 0:384], EXP, scale=SCALE)
            # PV with ones-augmented V: row 64 = softmax denominators
            o_ps = o_pair[:, hh * T : (hh + 1) * T]
            nc.tensor.matmul(
                o_ps,
                va_pair[0][:, h * 65 : h * 65 + 65],
                p_sb[:, 0:T],
                start=True,
                stop=False,
            )
            nc.tensor.matmul(
                o_ps[:, 128:T],
                va_pair[1][:, h * 65 : h * 65 + 65],
                p_sb[:, T : T + 128],
                start=False,
                stop=True,
            )
        # normalization for the pair: gather denominators (ACT), fast
        # reciprocal (DVE custom op, SBUF input), broadcast (GPSIMD),
        # then scale O^T while copying PSUM->SBUF (DVE)
        rg = rpool.tile([1, 2 * T], F32, tag="rg", name="rg")
        nc.scalar.copy(rg, o_pair[64:65, :])
        rr = rpool.tile([1, 2 * T], F32, tag="rr", name="rr")
        nc.vector.reciprocal_approx_fast(rr, rg)
        bc = bcpool.tile([64, 2 * T], F32, tag="bc", name="bc_sb")
        nc.gpsimd.partition_broadcast(bc, rr)
        for hh in range(2):
            nc.vector.tensor_mul(
                oT_sb[hp][hh * 64 : (hh + 1) * 64, :],
                o_pair[0:64, hh * T : (hh + 1) * T],
                bc[:, hh * T : (hh + 1) * T],
            )

    def emit_outproj(oT_sb, ib):
        for tt in range(2):
            yp = ps_y.tile([128, D], F32, tag="y", name="y_ps")
            for kc in range(3):
                nc.tensor.matmul(
                    yp,
                    oT_sb[kc][:, tt * 128 : (tt + 1) * 128],
                    wo_sb[kc],
                    start=(kc == 0),
                    stop=(kc == 2),
                )
            y_sb = ypool.tile([128, D], F32, tag="ysb", name="y_sb")
            nc.vector.tensor_add(y_sb, yp, bias_sb)
            nc.sync.dma_start(out=y[ib, tt * 128 : (tt + 1) * 128, :], in_=y_sb)

    def pop_pv():
        va_pair, oT_row, hp, sps, sub, mbi = pv_q.pop(0)
        emit_softmax_pv(va_pair, oT_row, hp, sps)
        if hp == 2:
            out_q.append((oT_row, 2 * mbi + sub))

    def pop_out():
        oT_row, ib = out_q.pop(0)
        emit_outproj(oT_row, ib)

    for mb in range(nmb):
        # ---- load xT for 2 batches: 3 d-chunk tiles [128, 512]
        x_sb = []
        for kc in range(3):
            t = xpool.tile([128, 2 * T], F16, tag="x", name="x")
            nc.sync.dma_start(
                out=t.rearrange("p (b t) -> p b t", t=T), in_=x2_src(mb, kc)
            )
            x_sb.append(t)

        # ---- q/k projections for both batches at once (N=512)
        qk_sb = {}
        for name, w_sb, use_act in (("q", wq_sb, True), ("k", wk_sb, False)):
            for mt in range(3):
                ps = ps_work.tile([128, 2 * T], F32, tag="work", name="qk_ps")
                for kc in range(3):
                    nc.tensor.matmul(
                        ps,
                        w_sb[kc][:, mt * 128 : (mt + 1) * 128],
                        x_sb[kc],
                        start=(kc == 0),
                        stop=(kc == 2),
                    )
                sb = qkpool.tile([128, 2 * T], F16, tag="qk", name="qk_sb")
                if use_act:
                    nc.scalar.copy(sb, ps)
                else:
                    nc.vector.tensor_copy(sb, ps)
                qk_sb[(name, mt)] = sb

        # ---- v projection per sub-batch -> persistent v_aug tiles
        va_mb = va_all[mb % 2]
        for sub in range(2):
            for st in range(2):
                ps = ps_work.tile([128, 2 * T], F32, tag="work", name="v_ps")[:, 0:D]
                for kc in range(3):
                    nc.tensor.matmul(
                        ps,
                        x_sb[kc][:, sub * T + st * 128 : sub * T + (st + 1) * 128],
                        wv_sb[kc],
                        start=(kc == 0),
                        stop=(kc == 2),
                    )
                va3 = va_mb[sub][st].rearrange("p (h c) -> p h c", c=65)
                nc.scalar.copy(va3[:, :, 0:64], ps.rearrange("p (h e) -> p h e", e=64))

        # ---- attention pipeline over (sub-batch, head-pair) stages.
        # Score matmuls lead their softmax/PV block by two stages (one at
        # the macro boundary), and each output projection trails its last
        # PV by one stage, so the PE never waits on the cross-engine
        # mask->exp->PV->normalize chains.
        oT = [
            [opool.tile([128, T], F16, tag="oT", name="oT") for _ in range(3)]
            for _ in range(2)
        ]
        for j, (sub, hp) in enumerate([(s, p) for s in range(2) for p in range(3)]):
            sps = emit_scores(qk_sb, sub, hp)
            pv_q.append((va_mb[sub], oT[sub], hp, sps, sub, mb))
            while len(pv_q) > (2 if j < 5 else 1):
                pop_pv()
                while len(out_q) > 1:
                    pop_out()

    # drain
    while pv_q:
        pop_pv()
    while out_q:
        pop_out()


def build_nc(bpc=BPC):
    nc = bacc.Bacc(
        "TRN2", target_bir_lowering=False, debug=False, enable_asserts=False
    )
    xT = nc.dram_tensor("xT", [bpc, D, T], F16, kind="ExternalInput").ap()
    wq = nc.dram_tensor("wq", [D, D], F16, kind="ExternalInput").ap()
    wk = nc.dram_tensor("wk", [D, D], F16, kind="ExternalInput").ap()
    wv = nc.dram_tensor("wv", [D, D], F16, kind="ExternalInput").ap()
    wo = nc.dram_tensor("wo", [D, D], F16, kind="ExternalInput").ap()
    bias = nc.dram_tensor("bias", [128, D], F32, kind="ExternalInput").ap()
    msk = nc.dram_tensor("msk", [128, 256], F32, kind="ExternalInput").ap()
    vone = nc.dram_tensor("vone", [128, H], F16, kind="ExternalInput").ap()
    y = nc.dram_tensor("y", [bpc, T, D], F32, kind="ExternalOutput").ap()
    with tile.TileContext(nc) as tc:
        with ExitStack() as ctx:
            _emit(ctx, tc, (xT, wq, wk, wv, wo, bias, msk, vone, y), bpc)
    nc.finalize()
    return nc


_NC_CACHE = {}


def _get_nc(bpc):
    if bpc not in _NC_CACHE:
        _NC_CACHE[bpc] = build_nc(bpc)
    return _NC_CACHE[bpc]


def prep_inputs(x, W_qkv, W_out, b_out):
    x = np.asarray(x, np.float32)
    W_qkv = np.asarray(W_qkv, np.float32)
    nb = x.shape[0]
    xT = np.ascontiguousarray(x.transpose(0, 2, 1)).astype(np.float16)
    Wq = np.ascontiguousarray(
        W_qkv[:, :, 0:64].transpose(1, 0, 2).reshape(D, D)
    ).astype(np.float16)
    Wk = np.ascontiguousarray(
        W_qkv[:, :, 64:128].transpose(1, 0, 2).reshape(D, D)
    ).astype(np.float16)
    Wv = np.ascontiguousarray(
        W_qkv[:, :, 128:192].transpose(1, 0, 2).reshape(D, D)
    ).astype(np.float16)
    Wo = np.ascontiguousarray(np.asarray(W_out, np.float32)).astype(np.float16)
    bias = np.ascontiguousarray(
        np.broadcast_to(np.asarray(b_out, np.float32), (128, D))
    )
    tri_neg = np.where(
        np.arange(128)[:, None] <= np.arange(128)[None, :], 0.0, -1e9
    ).astype(np.float32)
    msk = np.concatenate([tri_neg, tri_neg], axis=1)
    return xT, Wq, Wk, Wv, Wo, bias, msk, nb


def run(x, W_qkv, W_out, b_out, trace=False, **spmd_kwargs):
    xT, Wq, Wk, Wv, Wo, bias, msk, nb = prep_inputs(x, W_qkv, W_out, b_out)
    bpc = nb // NCORES
    assert bpc * NCORES == nb
    nc = _get_nc(bpc)
    shards = xT.reshape(NCORES, bpc, D, T)
    in_maps = [
        {
            "xT": shards[i],
            "wq": Wq,
            "wk": Wk,
            "wv": Wv,
            "wo": Wo,
            "bias": bias,
            "msk": msk,
            "vone": np.ones((128, H), np.float16),
        }
        for i in range(NCORES)
    ]
    res = run_bass_kernel_spmd(
        nc, in_maps, list(range(NCORES)), trace=trace, **spmd_kwargs
    )
    y = np.concatenate([res.results[i]["y"] for i in range(NCORES)], axis=0)
    return y, res


def kernel(x, W_qkv, W_out, b_out):
    y, _ = run(np.asarray(x), np.asarray(W_qkv), np.asarray(W_out), np.asarray(b_out))
    return y

